# revision 1
# baseline (speedup 1.0000x reference)
"""Trainium2 kernel for nn_CropRandomizer: batch of images -> N random crops each.

Strategy: pure data parallel over the batch (8 images per core, 8 cores).
Each of the 64 crops a core owns is a single DRAM->DRAM DMA whose source
access pattern [C, CROP_H, CROP_W] has a runtime register offset computed
from the crop indices (r*W + q + b*C*H*W). The two HWDGE engines (SP/ACT)
each issue half the crops, so descriptor generation runs on two rings.
No SBUF bounce for the pixel data: SDMA moves HBM->HBM directly.
"""
import numpy as np
from concourse import bass, bacc, mybir
from concourse.bass_utils import run_bass_kernel_spmd

M = 8  # cores
B, C, H, W = 64, 3, 224, 224
N = 8
CH = CW = 192
B_LOC = B // M  # images per core
U = B_LOC * N  # crops per core
MAXOFF = H - CH  # 32

_nc = None
LAST_RESULT = None


def _build():
    nc = bacc.Bacc()
    images = nc.dram_tensor(
        "images", [B_LOC, C, H, W], mybir.dt.float32, kind="ExternalInput"
    )
    crop_inds = nc.dram_tensor(
        "crop_inds", [B_LOC, N, 2], mybir.dt.int32, kind="ExternalInput"
    )
    out = nc.dram_tensor("out", [U, C, CH, CW], mybir.dt.float32, kind="ExternalOutput")

    CHW = C * H * W
    HW = H * W

    with (
        nc.sbuf_tensor("ci", [1, U * 2], mybir.dt.int32) as ci,
        nc.semaphore("in_sem") as in_sem,
        nc.semaphore("sp_sem") as sp_sem,
        nc.semaphore("act_sem") as act_sem,
        nc.Block() as block,
    ):

        def issue_crops(eng, crops, dsem):
            eng.wait_ge(in_sem, 16)
            tr = eng.alloc_register("off")
            tq = eng.alloc_register("col")
            ndma = 0
            for u in crops:
                b = u // N
                eng.reg_load([tr, tq], ci[:1, 2 * u : 2 * u + 2])
                eng.reg_mul(tr, tr, W)
                eng.reg_add(tr, tr, tq)
                if b:
                    eng.reg_add(tr, tr, b * CHW)
                src = bass.AP(images, tr, [[HW, C], [W, CH], [1, CW]])
                eng.dma_start(out[u], src).then_inc(dsem, 16)
                ndma += 1
            eng.wait_ge(dsem, 16 * ndma)

        @block.sync
        def _(sync):
            sync.dma_start(
                ci[:1, :], crop_inds.rearrange("b n t -> (b n t)")[None, :]
            ).then_inc(in_sem, 16)
            issue_crops(sync, range(0, U, 2), sp_sem)

        @block.scalar
        def _(scalar):
            issue_crops(scalar, range(1, U, 2), act_sem)

    nc.finalize()
    return nc


def kernel(images: np.ndarray, crop_inds: np.ndarray) -> np.ndarray:
    global _nc, LAST_RESULT
    if _nc is None:
        _nc = _build()
    images = np.ascontiguousarray(images, dtype=np.float32)
    crop_inds = np.ascontiguousarray(crop_inds, dtype=np.int32)
    in_maps = [
        {
            "images": images[m * B_LOC : (m + 1) * B_LOC],
            "crop_inds": crop_inds[m * B_LOC : (m + 1) * B_LOC],
        }
        for m in range(M)
    ]
    LAST_RESULT = run_bass_kernel_spmd(_nc, in_maps, core_ids=list(range(M)))
    return np.concatenate(
        [LAST_RESULT.results[m]["out"] for m in range(M)], axis=0
    )
